# revision 8
# baseline (speedup 1.0000x reference)
"""LocationAttention Trainium2 kernel (nn_LocationAttention_83485574300223).

out[b,t,:] = sum_{s<=t} a[b,s] x[b,s,:] / (sum_{s<=t} a[b,s] + eps),
a = exp(x @ w + b).

Strategy: data-parallel over batch (16 -> 2 per core, 8 cores). Per batch
element the causal prefix sums are computed with triangular matmuls on the
TensorEngine in float32r (TF32-class), 128-token chunks, superblocks of 4
chunks carried by a K=1 ones-broadcast matmul; den via a batch-level scan
pipeline. x*w reduce split DVE(mul)/ACT(accum).
"""
import numpy as np

import concourse.bass as bass
import concourse.tile as tile
from concourse import mybir
from concourse.bass_utils import run_bass_kernel_spmd

B, S, H = 16, 4096, 512
NCORES = 8
BPC = B // NCORES  # batch elements per core
P = 128            # partitions == chunk length
CPB = S // P       # chunks per batch element (32)
GF = 4             # chunks per group == superblock size
NG = CPB // GF     # groups per batch element (8)
EPS = 1e-9

F32 = mybir.dt.float32
F32R = mybir.dt.float32r
AF = mybir.ActivationFunctionType
ALU = mybir.AluOpType


def _split_multiwaits(nc, limit=1):
    """This walrus build accepts at most one sync-wait per instruction.
    Split extras into preceding single-wait NoOps on the same engine."""
    for fn in nc.m.functions:
        for bb in fn.blocks:
            out = []
            changed = False
            for ins in bb.instructions:
                si = getattr(ins, "sync_info", None)
                waits = list(si.on_wait) if (si is not None and si.on_wait) else []
                if len(waits) > limit:
                    extra, keep = waits[:-limit], waits[-limit:]
                    for i, w in enumerate(extra):
                        nop = mybir.InstNoOp(name=f"{ins.name}-ws{i}", ins=[], outs=[])
                        nop.engine = ins.engine
                        nop.sync_info = mybir.SyncInfo(on_wait=[w], on_update=[])
                        out.append(nop)
                    si.on_wait = keep
                    changed = True
                out.append(ins)
            if changed:
                try:
                    bb.instructions = out
                except Exception:
                    bb.instructions.clear()
                    bb.instructions.extend(out)


def _build():
    nc = bass.Bass()
    x = nc.declare_dram_parameter("x", [BPC, S, H], F32R, isOutput=False)
    wb4 = nc.declare_dram_parameter("wb4", [P, GF, H], F32, isOutput=False)
    tri = nc.declare_dram_parameter("tri", [P, P], F32, isOutput=False)
    bsc = nc.declare_dram_parameter("bsc", [1, 1], F32, isOutput=False)
    out = nc.declare_dram_parameter("out", [BPC, S, H], F32, isOutput=True)

    with tile.TileContext(nc) as tc:
        with (
            tc.tile_pool(name="singles", bufs=1) as singles,
            tc.tile_pool(name="xres", bufs=2 * NG) as xres,
            tc.tile_pool(name="xw", bufs=2) as xwp,
            tc.tile_pool(name="outp", bufs=2) as outp,
            tc.tile_pool(name="lhs", bufs=8) as lhsp,
            tc.tile_pool(name="smalls", bufs=2) as smalls,
            tc.tile_pool(name="carryp", bufs=2) as carryp,
            tc.tile_pool(name="nps", bufs=6, space="PSUM") as nps,
            tc.tile_pool(name="dps", bufs=2, space="PSUM") as dps,
        ):
            # ---- constants ----
            wb4_t = singles.tile([P, GF, H], F32)
            nc.sync.dma_start(out=wb4_t, in_=wb4[:])
            tri_t = singles.tile([P, P], F32)
            nc.sync.dma_start(out=tri_t, in_=tri[:])
            b_sb = singles.tile([P, 1], F32)
            nc.gpsimd.dma_start(out=b_sb, in_=bsc[:].to_broadcast([P, 1]))
            ones128 = singles.tile([P, P], F32)
            nc.vector.memset(ones128[:], 1.0)
            ones_row_f = singles.tile([1, P], F32)
            nc.vector.memset(ones_row_f[:], 1.0)
            ones_row_r = singles.tile([1, P], F32R)
            nc.vector.tensor_copy(ones_row_r[:], ones_row_f[:])
            eps_t = singles.tile([P, 1], F32)
            nc.vector.memset(eps_t[:], EPS)
            zeros_t = singles.tile([1, CPB], F32)
            nc.vector.memset(zeros_t[:], 0.0)

            for bi in range(BPC):
                xg = x[bi].rearrange("(g f p) h -> g p f h", p=P, f=GF)
                og = out[bi].rearrange("(g f p) h -> g p f h", p=P, f=GF)

                # ---- phase A: a = exp(x@w + b) ----
                p_all = smalls.tile([P, CPB], F32, tag="p_all")
                xts = []
                for g in range(NG):
                    xt = xres.tile([P, GF, H], F32R, tag="xg")
                    nc.sync.dma_start(out=xt, in_=xg[g])
                    xts.append(xt)
                    xw = xwp.tile([P, GF, H], F32, tag="xw")
                    nc.vector.tensor_mul(xw[:], xt[:].bitcast(F32), wb4_t[:])
                    for f in range(GF):
                        c = g * GF + f
                        nc.scalar.activation(
                            out=xw[:, f, :], in_=xw[:, f, :], func=AF.Copy,
                            accum_out=p_all[:, c : c + 1],
                        )

                a_all = smalls.tile([P, CPB], F32, tag="a_all")
                nc.scalar.activation(
                    out=a_all[:], in_=p_all[:], func=AF.Exp, bias=b_sb[:, 0:1]
                )

                # ---- den: r = 1/(cumsum(a) + eps), batch-level (fp32 PE) ----
                den_ps = dps.tile([P, CPB], F32, tag="den")
                nc.tensor.matmul(den_ps[:], tri_t[:], a_all[:], start=True, stop=False)
                # chunk-local cumsums to SBUF; last row (=chunk totals) via DMA
                den_loc = smalls.tile([P, CPB], F32, tag="den_loc")
                nc.scalar.activation(out=den_loc[:], in_=den_ps[:], func=AF.Copy)
                tot = smalls.tile([1, CPB], F32, tag="tot")
                nc.sync.dma_start(out=tot, in_=den_loc[127:128, :])
                excl = smalls.tile([1, CPB], F32, tag="excl")
                nc.vector.memset(excl[0:1, 0:1], 0.0)
                nc.vector.tensor_tensor_scan(
                    out=excl[0:1, 1:CPB],
                    data0=tot[0:1, 0 : CPB - 1],
                    data1=zeros_t[0:1, 0 : CPB - 1],
                    initial=0.0,
                    op0=ALU.add,
                    op1=ALU.add,
                )
                nc.tensor.matmul(den_ps[:], ones_row_f[:], excl[:], start=False, stop=True)
                den_sb = smalls.tile([P, CPB], F32, tag="den_sb")
                nc.scalar.activation(
                    out=den_sb[:], in_=den_ps[:], func=AF.Identity, bias=eps_t[:, 0:1]
                )
                r_all = smalls.tile([P, CPB], F32, tag="r_all")
                nc.vector.reciprocal(r_all[:], den_sb[:])
                # last-token den row (for carry un-scaling), via DMA
                den_last = smalls.tile([1, CPB], F32, tag="den_last")
                nc.sync.dma_start(out=den_last, in_=den_sb[127:128, :])

                # ---- phase B: num prefix sums + scale + store ----
                carry = None
                for k in range(NG):
                    xt = xts[k]
                    trias = []
                    for j in range(GF):
                        c = k * GF + j
                        tria = lhsp.tile([P, P], F32R, tag="tria")
                        nc.vector.tensor_scalar_mul(
                            tria[:], tri_t[:], a_all[:, c : c + 1]
                        )
                        trias.append(tria)
                    abcs = []
                    for i in range(GF - 1):
                        c = k * GF + i
                        abc = lhsp.tile([P, P], F32R, tag="abc")
                        nc.vector.tensor_scalar_mul(
                            abc[:], ones128[:], a_all[:, c : c + 1]
                        )
                        abcs.append(abc)

                    psums = [nps.tile([P, H], F32, tag="ps", name=f"ps_{bi}_{k}_{j}") for j in range(GF)]
                    # per-psum matmul list: tri_j, abcast_{i<j}, K1(carry)
                    mm_lists = []
                    for j in range(GF):
                        mms = [(trias[j][:], xt[:, j, :])]
                        for i in range(j):
                            mms.append((abcs[i][:], xt[:, i, :]))
                        if carry is not None:
                            mms.append((ones_row_r[:], carry[:]))
                        mm_lists.append(mms)
                    for j in range(GF):
                        n = len(mm_lists[j])
                        for m, (lhsT, rhs) in enumerate(mm_lists[j]):
                            nc.tensor.matmul(
                                psums[j][:], lhsT, rhs,
                                start=(m == 0), stop=(m == n - 1),
                            )

                    og_t = outp.tile([P, GF, H], F32, tag="og")
                    for j in range(GF):
                        c = k * GF + j
                        nc.scalar.activation(
                            out=og_t[:, j, :], in_=psums[j][:], func=AF.Copy,
                            scale=r_all[:, c : c + 1],
                        )
                    nc.sync.dma_start(out=og[k], in_=og_t)

                    # carry for next superblock: last scaled row, un-scaled by
                    # den (row read via SBUF->SBUF DMA; engines can't address
                    # partition 127 directly)
                    if k < NG - 1:
                        cend = k * GF + GF - 1
                        crow = carryp.tile([1, H], F32, tag="crow")
                        nc.sync.dma_start(out=crow, in_=og_t[127:128, GF - 1, :])
                        new_carry = carryp.tile([1, H], F32R, tag="carry")
                        nc.vector.tensor_scalar_mul(
                            new_carry[:], crow[:], den_last[0:1, cend : cend + 1]
                        )
                        carry = new_carry

    _split_multiwaits(nc)
    return nc


_NC = None


def _get_nc():
    global _NC
    if _NC is None:
        _NC = _build()
    return _NC


def _prep_in_maps(input_data, w, b):
    x = np.ascontiguousarray(np.asarray(input_data, dtype=np.float32))
    assert x.shape == (B, S, H), x.shape
    w = np.asarray(w, dtype=np.float32).reshape(H)
    b = np.float32(np.asarray(b, dtype=np.float32).reshape(()))
    wb4 = np.ascontiguousarray(np.broadcast_to(w, (P, GF, H)), dtype=np.float32)
    tri = np.triu(np.ones((P, P), dtype=np.float32))
    bsc = np.full((1, 1), b, dtype=np.float32)
    return [
        {
            "x": np.ascontiguousarray(x[i * BPC : (i + 1) * BPC]),
            "wb4": wb4,
            "tri": tri,
            "bsc": bsc,
        }
        for i in range(NCORES)
    ]


def _run(input_data, w, b, trace=False):
    nc = _get_nc()
    in_maps = _prep_in_maps(input_data, w, b)
    res = run_bass_kernel_spmd(
        nc, in_maps, core_ids=list(range(NCORES)), trace=trace
    )
    out = np.concatenate([res.results[i]["out"] for i in range(NCORES)], axis=0)
    return out.astype(np.float32, copy=False), res


def kernel(input_data, w, b):
    out, _ = _run(input_data, w, b, trace=False)
    return out


# revision 9
# speedup vs baseline: 1.1007x; 1.1007x over previous
"""LocationAttention Trainium2 kernel (nn_LocationAttention_83485574300223).

out[b,t,:] = sum_{s<=t} a[b,s] x[b,s,:] / (sum_{s<=t} a[b,s] + eps),
a = exp(x @ w + b).

Data-parallel over batch: 16 -> 2 per core, 8 cores. Per batch element the
causal prefix sums run as triangular matmuls on the TensorEngine in float32r
(TF32-class, full rate at N=512): 128-token chunks, superblocks of 4 chunks.
Inter-superblock carry: last output row re-scaled by den (row extracted with
a small SBUF->SBUF DMA - engines cannot address partition 127) feeding a K=1
ones-broadcast matmul. den: per-superblock column-sum matmul + scan chain.
Single streaming pipeline, no batch-level barriers.
"""
import numpy as np

import concourse.bass as bass
import concourse.tile as tile
from concourse import mybir
from concourse.bass_utils import run_bass_kernel_spmd

B, S, H = 16, 4096, 512
NCORES = 8
BPC = B // NCORES  # batch elements per core
P = 128            # partitions == chunk length
CPB = S // P       # chunks per batch element (32)
GF = 4             # chunks per superblock
NG = CPB // GF     # superblocks per batch element (8)
EPS = 1e-9

F32 = mybir.dt.float32
F32R = mybir.dt.float32r
AF = mybir.ActivationFunctionType
ALU = mybir.AluOpType
AX = mybir.AxisListType


def _split_multiwaits(nc, limit=1):
    """This walrus build accepts at most one sync-wait per instruction.
    Split extras into preceding single-wait NoOps on the same engine."""
    for fn in nc.m.functions:
        for bb in fn.blocks:
            out = []
            changed = False
            for ins in bb.instructions:
                si = getattr(ins, "sync_info", None)
                waits = list(si.on_wait) if (si is not None and si.on_wait) else []
                if len(waits) > limit:
                    extra, keep = waits[:-limit], waits[-limit:]
                    for i, w in enumerate(extra):
                        nop = mybir.InstNoOp(name=f"{ins.name}-ws{i}", ins=[], outs=[])
                        nop.engine = ins.engine
                        nop.sync_info = mybir.SyncInfo(on_wait=[w], on_update=[])
                        out.append(nop)
                    si.on_wait = keep
                    changed = True
                out.append(ins)
            if changed:
                try:
                    bb.instructions = out
                except Exception:
                    bb.instructions.clear()
                    bb.instructions.extend(out)


def _build():
    nc = bass.Bass()
    x = nc.declare_dram_parameter("x", [BPC, S, H], F32R, isOutput=False)
    wb4 = nc.declare_dram_parameter("wb4", [P, GF, H], F32, isOutput=False)
    tri = nc.declare_dram_parameter("tri", [P, P], F32, isOutput=False)
    bsc = nc.declare_dram_parameter("bsc", [1, 1], F32, isOutput=False)
    out = nc.declare_dram_parameter("out", [BPC, S, H], F32, isOutput=True)

    with tile.TileContext(nc) as tc:
        with (
            tc.tile_pool(name="singles", bufs=1) as singles,
            tc.tile_pool(name="xp", bufs=4) as xp,
            tc.tile_pool(name="xwp", bufs=3) as xwp,
            tc.tile_pool(name="outp", bufs=3) as outp,
            tc.tile_pool(name="lhsp", bufs=8) as lhsp,
            tc.tile_pool(name="smallp", bufs=3) as smallp,
            tc.tile_pool(name="carryp", bufs=2) as carryp,
            tc.tile_pool(name="nps", bufs=6, space="PSUM") as nps,
            tc.tile_pool(name="dps", bufs=2, space="PSUM") as dps,
        ):
            # ---- constants ----
            wb4_t = singles.tile([P, GF, H], F32)
            nc.sync.dma_start(out=wb4_t, in_=wb4[:])
            tri_t = singles.tile([P, P], F32)
            nc.sync.dma_start(out=tri_t, in_=tri[:])
            b_sb = singles.tile([P, 1], F32)
            nc.gpsimd.dma_start(out=b_sb, in_=bsc[:].to_broadcast([P, 1]))
            ones128 = singles.tile([P, P], F32)
            nc.vector.memset(ones128[:], 1.0)
            ones_row_f = singles.tile([1, P], F32)
            nc.vector.memset(ones_row_f[:], 1.0)
            ones_row_r = singles.tile([1, P], F32R)
            nc.vector.tensor_copy(ones_row_r[:], ones_row_f[:])
            eps_t = singles.tile([P, 1], F32)
            nc.vector.memset(eps_t[:], EPS)
            zeros_t = singles.tile([1, CPB], F32)
            nc.vector.memset(zeros_t[:], 0.0)

            for bi in range(BPC):
                xg = x[bi].rearrange("(g f p) h -> g p f h", p=P, f=GF)
                og = out[bi].rearrange("(g f p) h -> g p f h", p=P, f=GF)

                carry = None       # [1, H] f32r, running num prefix total
                prev_dexcl = None  # [1, GF+1] f32, col GF = running den total
                for k in range(NG):
                    # -- load + x@w partial products --
                    xt = xp.tile([P, GF, H], F32R, tag="xt", name=f"xt_{bi}_{k}")
                    nc.sync.dma_start(out=xt, in_=xg[k])
                    xw = xwp.tile([P, GF, H], F32, tag="xw", name=f"xw_{bi}_{k}")
                    nc.vector.tensor_mul(xw[:], xt[:].bitcast(F32), wb4_t[:])
                    p4 = smallp.tile([P, GF], F32, tag="p4", name=f"p4_{bi}_{k}")
                    for f in range(GF):
                        if f < 2:  # reduces split ACT/DVE for balance
                            nc.scalar.activation(
                                out=xw[:, f, :], in_=xw[:, f, :], func=AF.Copy,
                                accum_out=p4[:, f : f + 1],
                            )
                        else:
                            nc.vector.tensor_reduce(
                                out=p4[:, f : f + 1], in_=xw[:, f, :],
                                axis=AX.X, op=ALU.add,
                            )
                    a4 = smallp.tile([P, GF], F32, tag="a4", name=f"a4_{bi}_{k}")
                    nc.scalar.activation(
                        out=a4[:], in_=p4[:], func=AF.Exp, bias=b_sb[:, 0:1]
                    )

                    # -- den for this superblock --
                    den_ps = dps.tile([P, 2 * GF], F32, tag="den", name=f"dps_{bi}_{k}")
                    # chunk totals (col-sums of a4) into cols GF..2GF
                    nc.tensor.matmul(
                        den_ps[0:1, GF : 2 * GF], ones128[:, 0:1], a4[:],
                        start=True, stop=True,
                    )
                    # chunk-local cumsums into cols 0..GF
                    nc.tensor.matmul(
                        den_ps[:, 0:GF], tri_t[:], a4[:],
                        start=True, stop=False, skip_group_check=True,
                    )
                    # exclusive-prefix chain across chunks/superblocks
                    dexcl = smallp.tile([1, GF + 1], F32, tag="dexcl", name=f"dex_{bi}_{k}")
                    if prev_dexcl is None:
                        nc.vector.memset(dexcl[0:1, 0:1], 0.0)
                    else:
                        nc.vector.tensor_copy(
                            dexcl[0:1, 0:1], prev_dexcl[0:1, GF : GF + 1]
                        )
                    nc.vector.tensor_tensor_scan(
                        out=dexcl[0:1, 1 : GF + 1],
                        data0=den_ps[0:1, GF : 2 * GF],
                        data1=zeros_t[0:1, 0:GF],
                        initial=dexcl[0:1, 0:1],
                        op0=ALU.add,
                        op1=ALU.add,
                    )
                    prev_dexcl = dexcl
                    nc.tensor.matmul(
                        den_ps[:, 0:GF], ones_row_f[:], dexcl[0:1, 0:GF],
                        start=False, stop=True,
                    )
                    den4 = smallp.tile([P, GF], F32, tag="den4", name=f"den4_{bi}_{k}")
                    nc.scalar.activation(
                        out=den4[:], in_=den_ps[:, 0:GF], func=AF.Identity,
                        bias=eps_t[:, 0:1],
                    )
                    r4 = smallp.tile([P, GF], F32, tag="r4", name=f"r4_{bi}_{k}")
                    nc.vector.reciprocal(r4[:], den4[:])
                    den_row = smallp.tile([1, GF], F32, tag="den_row", name=f"drow_{bi}_{k}")
                    nc.gpsimd.dma_start(out=den_row, in_=den4[127:128, :])

                    # -- num lhsT builds --
                    trias = []
                    for j in range(GF):
                        tria = lhsp.tile([P, P], F32R, tag="tria", name=f"tria_{bi}_{k}_{j}")
                        nc.vector.tensor_scalar_mul(
                            tria[:], tri_t[:], a4[:, j : j + 1]
                        )
                        trias.append(tria)
                    abcs = []
                    for i in range(GF - 1):
                        abc = lhsp.tile([P, P], F32R, tag="abc", name=f"abc_{bi}_{k}_{i}")
                        nc.vector.tensor_scalar_mul(
                            abc[:], ones128[:], a4[:, i : i + 1]
                        )
                        abcs.append(abc)

                    # -- num matmuls --
                    psums = [
                        nps.tile([P, H], F32, tag="ps", name=f"ps_{bi}_{k}_{j}")
                        for j in range(GF)
                    ]
                    for j in range(GF):
                        mms = [(trias[j][:], xt[:, j, :])]
                        for i in range(j):
                            mms.append((abcs[i][:], xt[:, i, :]))
                        if carry is not None:
                            mms.append((ones_row_r[:], carry[:]))
                        n = len(mms)
                        for m, (lhsT, rhs) in enumerate(mms):
                            nc.tensor.matmul(
                                psums[j][:], lhsT, rhs,
                                start=(m == 0), stop=(m == n - 1),
                            )

                    # -- scale + store --
                    og_t = outp.tile([P, GF, H], F32, tag="og", name=f"og_{bi}_{k}")
                    for j in range(GF):
                        nc.scalar.activation(
                            out=og_t[:, j, :], in_=psums[j][:], func=AF.Copy,
                            scale=r4[:, j : j + 1],
                        )
                    nc.sync.dma_start(out=og[k], in_=og_t)

                    # -- carry for next superblock --
                    if k < NG - 1:
                        crow = carryp.tile([1, H], F32, tag="crow", name=f"crow_{bi}_{k}")
                        nc.gpsimd.dma_start(out=crow, in_=og_t[127:128, GF - 1, :])
                        new_carry = carryp.tile([1, H], F32R, tag="carry", name=f"carry_{bi}_{k}")
                        nc.vector.tensor_scalar_mul(
                            new_carry[:], crow[:], den_row[0:1, GF - 1 : GF]
                        )
                        carry = new_carry
                    else:
                        carry = None

    _split_multiwaits(nc)
    return nc


_NC = None


def _get_nc():
    global _NC
    if _NC is None:
        _NC = _build()
    return _NC


def _prep_in_maps(input_data, w, b):
    x = np.ascontiguousarray(np.asarray(input_data, dtype=np.float32))
    assert x.shape == (B, S, H), x.shape
    w = np.asarray(w, dtype=np.float32).reshape(H)
    b = np.float32(np.asarray(b, dtype=np.float32).reshape(()))
    wb4 = np.ascontiguousarray(np.broadcast_to(w, (P, GF, H))).astype(np.float32)
    tri = np.triu(np.ones((P, P), dtype=np.float32))
    bsc = np.full((1, 1), b, dtype=np.float32)
    return [
        {
            "x": np.ascontiguousarray(x[i * BPC : (i + 1) * BPC]),
            "wb4": wb4,
            "tri": tri,
            "bsc": bsc,
        }
        for i in range(NCORES)
    ]


def _run(input_data, w, b, trace=False):
    nc = _get_nc()
    in_maps = _prep_in_maps(input_data, w, b)
    res = run_bass_kernel_spmd(
        nc, in_maps, core_ids=list(range(NCORES)), trace=trace
    )
    out = np.concatenate([res.results[i]["out"] for i in range(NCORES)], axis=0)
    return out.astype(np.float32, copy=False), res


def kernel(input_data, w, b):
    out, _ = _run(input_data, w, b, trace=False)
    return out
